# revision 7
# baseline (speedup 1.0000x reference)
"""Expert-parallel sparse MoE kernel for Trainium2 (8 NeuronCores).

Reference model: dense MoE (every expert on every token) followed by a
top-2-sparse combine, residual add, and LayerNorm.  Mathematically only the
top-2 experts per token contribute to the output, so the kernel routes each
token to its top-2 experts and only computes those expert FFNs.

Sharding: expert-parallel.  Each of the 8 cores owns 8 of the 64 experts and
receives the tokens routed to them (all-to-all by routing, done host-side as
part of sharding).  The device streams the expert weights (the dominant
memory traffic, quantized to fp8) and computes y_e = relu(x @ W1[e] + b1[e])
@ W2[e] for every routed token.  The host applies the gate weights + b2
during the unshard/scatter, adds the residual, and normalizes.

v7 (from the 74us baseline; v2/v3 were 85/89us experiments):
- fp8 e4m3 activations + weights: every matmul runs in DoubleRow mode (256
  contraction rows per pass, 2x PE throughput; baseline PE busy was 61us).
- all expert weights SBUF-resident (16.8MB), every weight DMA issued
  up-front in large chunks split across the SWDGE (gpsimd) and HWDGE (sync)
  rings -- the stream never stalls on compute (v2 measured all weights
  delivered by ~50us vs baseline 60us).
- v2 lesson: with DoubleRow, mm1 is so fast the in-order PE queue
  head-of-line blocks on mm2 -> relu -> DVE each h-pair, draining one
  expert per ~5us.  v3 interleaves expert i-1's mm2 chain into expert i's
  mm1 stream (ht/psum double-buffered), so mm2 operands are always long
  ready, and fuses relu to one op per h-pair (b1 == 0 here; general path
  kept for nonzero b1).  y DMAs ride the gpsimd ring to keep the Scalar
  queue free for relus.
"""

import numpy as np
import ml_dtypes

B, S, D, H, E, TOPK = 2, 1024, 512, 2048, 64, 2
T = B * S
NCORES = 8
EPC = E // NCORES          # experts per core
CAP = 96                   # token capacity per expert (observed max 95;
                           # overflow tokens fall back to exact host compute)
DC = D // 128              # 4 contraction chunks for x @ W1
HC = H // 128              # 16 contraction chunks for h @ W2
F1 = DC * H                # w1 elements per partition row (8192)
F2 = HC * D                # w2 elements per partition row (8192)
F = F1 + F2                # fused w1|w2 elements per partition row
EPS = 1e-5
BF16 = ml_dtypes.bfloat16

PROFILE = False            # set True (module-level) to capture an NTFF trace
LAST_RESULT = None         # BassKernelResults of the last run (for test.py)

# fp8 everywhere: W1/W2 are scaled by WSCALE on the host and stored e4m3;
# the 1/WSCALE^2 descale folds into the host-side combine.  x and the hidden
# activations h are also e4m3 (|x| < 6, |h_scaled| < 50, both well inside the
# TRN e4m3 range of +-240) which enables DoubleRow matmuls.
WSCALE = 16.0
FP8 = ml_dtypes.float8_e4m3fn

_NC_CACHE = {}


def _build_bass(fused_relu):
    """Build the per-core Bass/Tile program (identical on all 8 cores).

    fused_relu: relu two h-chunks per op (requires b1 == 0, since the
    activation bias operand is a per-partition scalar and cannot vary
    along the fused pair).
    """
    import concourse.bacc as bacc
    import concourse.mybir as mybir
    from concourse import tile

    nc = bacc.Bacc("TRN2", target_bir_lowering=False, debug=False,
                   num_devices=NCORES)

    bf = mybir.dt.bfloat16
    f32 = mybir.dt.float32
    fp8 = mybir.dt.float8e4
    DR = mybir.MatmulPerfMode.DoubleRow
    xt = nc.dram_tensor("xt", [128, EPC, DC, CAP], fp8, kind="ExternalInput")
    # W1|W2 fused per expert: [d-part, DC*H (w1) + HC*D (w2)]
    w12 = nc.dram_tensor("w12", [EPC, 128, F], fp8, kind="ExternalInput")
    b1 = nc.dram_tensor("b1", [128, EPC, HC], f32, kind="ExternalInput")
    y = nc.dram_tensor("y", [EPC, CAP, D], bf, kind="ExternalOutput")

    relu = mybir.ActivationFunctionType.Relu
    alu_add = mybir.AluOpType.add
    alu_max = mybir.AluOpType.max

    with tile.TileContext(nc) as tc:
        with (
            tc.tile_pool(name="acts", bufs=2) as acts,
            tc.tile_pool(name="yts", bufs=EPC) as yts,
            tc.tile_pool(name="cst", bufs=1) as cst,
            tc.tile_pool(name="ps1", bufs=4, space="PSUM") as ps1,
            tc.tile_pool(name="ps2", bufs=2, space="PSUM") as ps2,
            tc.tile_pool(name="psd", bufs=1, space="PSUM") as psd,
        ):
            # Tokens + biases ride the sync ring first; a dummy ReLU reading
            # them advances the ACT engine past the DMA sem and pays the
            # activation-table load once, so steady-state Activations carry
            # only their PSUM wait (the ISA allows very few waits per ACT).
            xtt = cst.tile([128, EPC, DC, CAP], fp8, name="xtt")
            b1t = cst.tile([128, EPC, HC], f32, name="b1t")
            nc.gpsimd.dma_start(b1t[:], b1[:])
            nc.gpsimd.dma_start(xtt[:], xt[:])
            scratch = cst.tile([128, 1], f32, name="scratch")
            nc.scalar.activation(scratch[:], b1t[:, 0, 0:1], relu,
                                 bias=b1t[:, 0, 0:1])
            # warm-keeper source: the PE HAM clock-gate drops to 1.2GHz after
            # ~3.4us idle; dummy DoubleRow matmuls on this zero tile keep the
            # PE busy through known DMA-wait gaps so real matmuls run at
            # 2.4GHz (a cold stretch costs ~4-6us across the kernel)
            dm = cst.tile([128, 2, CAP], fp8, name="dm")
            nc.vector.memset(dm[:], 0.0)

            # All weight DMAs up-front into one fully-resident tile (never
            # recycled): the HBM stream never waits on compute.  Chunks are
            # split across the gpsimd (SWDGE, ~310-410 GB/s) and sync
            # (HWDGE, ~215 GB/s) rings, sized so both finish together; the
            # very last dependency (second half of e7's w2) is a small
            # 0.5MB piece.  Subtile dependency tracking lets each expert's
            # matmuls wait only on the chunk covering its region.
            w_all = cst.tile([128, EPC, F], fp8, name="w_all")

            def wsrc(lo_e, hi_e, lo_f, hi_f):
                src = w12[lo_e:hi_e, :, lo_f:hi_f]
                return src.rearrange("e p f -> p e f")

            # All weights ride the SWDGE (gpsimd) ring alone: it sustains
            # ~340-410 GB/s solo, while the HWDGE (sync) ring starves to
            # <100 GB/s when contending with it (v3 lesson: an early expert
            # on sync stalled the PE 30us).  First/last experts are 1MB
            # pieces for pipeline startup/drain; the middle is 4.2MB chunks
            # for full-rate descriptors runs.
            nc.gpsimd.dma_start(w_all[:, 0, :F1 // 2], wsrc(0, 1, 0, F1 // 2))
            nc.gpsimd.dma_start(w_all[:, 0, F1 // 2:F1], wsrc(0, 1, F1 // 2, F1))
            nc.gpsimd.dma_start(w_all[:, 0, F1:], wsrc(0, 1, F1, F))
            nc.gpsimd.dma_start(w_all[:, 1:3], wsrc(1, 3, 0, F))
            nc.gpsimd.dma_start(w_all[:, 3:5], wsrc(3, 5, 0, F))
            nc.gpsimd.dma_start(w_all[:, 5:7], wsrc(5, 7, 0, F))
            nc.gpsimd.dma_start(w_all[:, 7, :F1], wsrc(7, 8, 0, F1))
            nc.gpsimd.dma_start(w_all[:, 7, F1:F1 + F2 // 2],
                                wsrc(7, 8, F1, F1 + F2 // 2))
            nc.gpsimd.dma_start(w_all[:, 7, F1 + F2 // 2:],
                                wsrc(7, 8, F1 + F2 // 2, F))

            def w1ap(i):
                return w_all[:, i, :F1].rearrange("p (c h) -> p c h", c=DC)

            def w2ap(i):
                return w_all[:, i, F1:].rearrange("p (c dd) -> p c dd", c=HC)

            # Steady state: PE streams expert i's mm1 pairs with expert
            # i-1's mm2 chain interleaved (its ht is long complete, so the
            # PE never waits on the relu engines); relus for pair jp
            # alternate ScalarE/VectorE.  DoubleRow everywhere.
            def mm2_step(i, jp):
                """mm2 for expert i, h-pair jp, into its p2; emit y at end."""
                j = 2 * jp
                nc.tensor.matmul(p2s[i][:], hts[i][:, j:j + 2, :],
                                 w2ap(i)[:, j:j + 2, :],
                                 start=(jp == 0), stop=(jp == HC // 2 - 1),
                                 perf_mode=DR, skip_group_check=True)
                if jp == HC // 2 - 1:
                    yt = yts.tile([CAP, D], bf, name="yt")
                    nc.vector.tensor_copy(yt[:], p2s[i][:])
                    nc.sync.dma_start(y[i], yt[:])

            pd = psd.tile([CAP, CAP], f32, name="pd")

            def warm(n):
                for _ in range(n):
                    nc.tensor.matmul(pd[:], dm[:], dm[:], start=True,
                                     stop=True, perf_mode=DR,
                                     skip_group_check=True)

            # gap sizes below are tuned to the measured chunk-arrival gaps;
            # a block that outlives its gap only costs its own ~40-60ns/op
            WARMN = {0: 80, 1: 24, 3: 16, 5: 16}
            hts = {}
            p2s = {}
            for i in range(EPC):
                hts[i] = acts.tile([128, HC, CAP], fp8, name="ht")
                p2s[i] = ps2.tile([CAP, D], f32, name="p2")
                if i > 0:
                    for jp in range(HC // 2):
                        mm2_step(i - 1, jp)
                warm(WARMN.get(i, 0))
                w1t = w1ap(i)
                for jp in range(HC // 2):
                    j = 2 * jp
                    p1 = ps1.tile([128, 2, CAP], f32, name="p1")
                    for jj in range(2):
                        for c in range(DC // 2):
                            nc.tensor.matmul(
                                p1[:, jj, :],
                                w1t[:, 2 * c:2 * c + 2,
                                    (j + jj) * 128:(j + jj + 1) * 128],
                                xtt[:, i, 2 * c:2 * c + 2, :],
                                start=(c == 0),
                                stop=(c == DC // 2 - 1),
                                perf_mode=DR,
                            )
                    if fused_relu:
                        if jp % 2 == 0:
                            nc.scalar.activation(hts[i][:, j:j + 2, :],
                                                 p1[:], relu)
                        else:
                            nc.vector.tensor_scalar(
                                hts[i][:, j:j + 2, :], p1[:], 0.0, 0.0,
                                alu_add, alu_max)
                    else:
                        for jj in range(2):
                            if (j + jj) % 2 == 0:
                                nc.scalar.activation(
                                    hts[i][:, j + jj, :], p1[:, jj, :], relu,
                                    bias=b1t[:, i, j + jj:j + jj + 1])
                            else:
                                nc.vector.tensor_scalar(
                                    hts[i][:, j + jj, :], p1[:, jj, :],
                                    b1t[:, i, j + jj:j + jj + 1], 0.0,
                                    alu_add, alu_max)
            for jp in range(HC // 2):  # drain the last expert
                mm2_step(EPC - 1, jp)

    # Bacc lowering: splits excess per-instruction sem waits onto
    # InstEventSemaphore, moves matmul waits onto ldweights, inserts
    # activation table loads -- required for walrus codegen (1 wait slot
    # per 64B ISA instruction).
    nc.compile()
    return nc


def _get_nc(fused_relu):
    key = ("nc", fused_relu)
    if key not in _NC_CACHE:
        _NC_CACHE[key] = _build_bass(fused_relu)
    return _NC_CACHE[key]


def kernel(x, Wg, bg, W1, b1, W2, b2, gamma, beta):
    global LAST_RESULT
    x = np.asarray(x, np.float32)
    Wg = np.asarray(Wg, np.float32)
    bg = np.asarray(bg, np.float32)
    W1 = np.asarray(W1, np.float32)
    b1 = np.asarray(b1, np.float32)
    W2 = np.asarray(W2, np.float32)
    b2 = np.asarray(b2, np.float32)
    gamma = np.asarray(gamma, np.float32)
    beta = np.asarray(beta, np.float32)

    xf = x.reshape(T, D)

    # ---- gating: softmax over experts, top-2 (ties -> lower index, as top_k)
    logits = xf @ Wg + bg
    logits -= logits.max(-1, keepdims=True)
    probs = np.exp(logits)
    probs /= probs.sum(-1, keepdims=True)
    idx = np.argsort(-probs, axis=-1, kind="stable")[:, :TOPK]   # [T, K]
    vals = np.take_along_axis(probs, idx, axis=-1)               # [T, K]

    # ---- per-expert token lists (the all-to-all "sharding by routing")
    slot = np.full((T, TOPK), -1, np.int64)
    toks_per_e = []
    overflow = []  # (expert, token_ids) pairs beyond CAP -> host fallback
    for e in range(E):
        te = np.nonzero((idx == e).any(-1))[0]
        if len(te) > CAP:
            overflow.append((e, te[CAP:]))
            te = te[:CAP]
        toks_per_e.append(te)
        if len(te):
            k_of = (idx[te] == e).argmax(-1)
            slot[te, k_of] = np.arange(len(te))

    # ---- pack per-core device inputs (layouts match SBUF tiles exactly)
    xth = np.zeros((E, 128, DC, CAP), FP8)
    for e in range(E):
        te = toks_per_e[e]
        if len(te):
            blk = xf[te].T.reshape(DC, 128, len(te)).transpose(1, 0, 2)
            xth[e, :, :, :len(te)] = blk.astype(FP8)
    wq = lambda a: (a * WSCALE).astype(FP8)
    w1h = wq(W1).reshape(E, DC, 128, H).transpose(0, 2, 1, 3)
    w2h = wq(W2).reshape(E, HC, 128, D).transpose(0, 2, 1, 3)
    w12h = np.concatenate([w1h.reshape(E, 128, DC * H),
                           w2h.reshape(E, 128, HC * D)], axis=2)
    b1s = b1 * WSCALE
    b1h = np.ascontiguousarray(b1s.reshape(E, HC, 128).transpose(0, 2, 1))

    in_maps = []
    for c in range(NCORES):
        sl = slice(c * EPC, (c + 1) * EPC)
        in_maps.append({
            "xt": np.ascontiguousarray(xth[sl].transpose(1, 0, 2, 3)),
            "w12": w12h[sl],
            "b1": np.ascontiguousarray(b1h[sl].transpose(1, 0, 2)),
        })

    # ---- run on the 8 cores
    from concourse.bass_utils import run_bass_kernel_spmd
    nc = _get_nc(fused_relu=not b1.any())
    res = run_bass_kernel_spmd(nc, in_maps, list(range(NCORES)),
                               trace=PROFILE)
    LAST_RESULT = res
    y_all = np.concatenate([r["y"] for r in res.results],
                           axis=0).astype(np.float32)             # [E,CAP,D]
    y_all /= WSCALE * WSCALE

    # ---- unshard: scatter expert outputs back by routing, combine, LN
    ok = slot >= 0
    sl = np.where(ok, slot, 0)
    contrib = y_all[idx, sl] + b2[idx]                 # [T, K, D]
    out = xf + (vals[..., None] * contrib * ok[..., None]).sum(1)

    for e, te in overflow:  # practically never taken (CAP >> max count)
        k_of = (idx[te] == e).argmax(-1)
        w = vals[te, k_of]
        h = np.maximum(xf[te] @ W1[e] + b1[e], 0.0)
        out[te] += w[:, None] * (h @ W2[e] + b2[e])

    mu = out.mean(-1, keepdims=True)
    var = ((out - mu) ** 2).mean(-1, keepdims=True)
    o = (out - mu) / np.sqrt(var + EPS) * gamma + beta
    return o.reshape(B, S, D).astype(np.float32)
